# revision 1
# baseline (speedup 1.0000x reference)
"""Distributed Trainium2 (Bass/Tile) kernel for AdaptiveGCNLayer.

Reference semantics (N=4096 nodes, C=512 channels):
    adj   = x @ W_adj @ x.T + I                      [N, N]
    adj   = d^-1/2 * adj * d^-1/2   (row sums d)     -- values then DISCARDED:
    A     = (adj != 0) with forced unit diagonal     (dense_to_sparse keeps only
                                                      the nonzero pattern)
    deg   = A.sum(1); dis = deg^-1/2 (0 if deg<=0)
    out   = (dis[:,None] * A * dis[None,:]) @ (x @ W_gcn) + b

Scaling rows/cols by nonzero (or NaN/inf) factors never changes the !=0
pattern, so A == (x @ W_adj @ x.T != 0) except on the measure-zero event of
an exactly-zero f32 entry; the first normalization is therefore not
materialized, and the adjacency can be computed at any precision (fp8 here)
since only its zero pattern survives.  deg >= 1 always (forced diagonal).

Sharding (8 cores, 1-D node partition, R=512 rows each): core i computes its
adjacency block in TRANSPOSED layout adjT [N, R] (directly usable as the
stationary operand of the final aggregation), masks it to {0,1} bf16,
reduces mask -> deg for its rows (ones-matmul on the TensorEngine),
AllGathers xg = x @ W_gcn (triggered early) and deg (the cross-core
"column degree" exchange), scales the gathered xg by dis, and aggregates:
out_rows = dis_r * (A_rows @ (dis * xg)) + b, bf16 matmuls with fp32 PSUM
accumulation.

Overlap / latency structure (this environment has a ~25-45us rank-dispatch
skew barrier on the first collective and ~11us collective-stream start
latency per op):
  - the xg AllGather is triggered ~20us in, so its ~26us wire time runs
    under the skew barrier + adjacency phase; the 16KB deg AllGather
    follows it immediately on the collective stream
  - adjacency matmuls run fp8e4m3 DoubleRow (only the zero pattern of the
    adjacency survives, so precision there is free)
  - the mask computation is split DVE(not_equal):ACT(sign^2) 2:1 — a single
    engine would pace the whole adjacency phase
  - y readbacks ride the sync queue exclusively and the deg bounce rides
    gpsimd: a queue that also carries later compute would hit these
    gather-gated DMA waits early (Tile reorders DMA ring entries) and
    freeze that compute for tens of us
  - the deg payload is written partition-major so the post-gather readbacks
    are fast contiguous reads
  - the final aggregation is m-outer so each PSUM bank accumulates a long
    33-matmul chain (per-matmul bank-cycling triggers HAM oscillation)
  - the bias enters through a rank-1 matmul sqrt(deg_r) (x) bias folded into
    the same PSUM accumulation (it cancels the later dis_r row scaling), so
    no broadcast tile or extra elementwise pass is needed
"""

import numpy as np

from concourse import bacc, mybir, tile
from concourse.bass_utils import run_bass_kernel_spmd

N_CORES = 8
N = 4096               # nodes
C = 512                # channels (C_IN == C_OUT)
R = N // N_CORES       # 512 rows per core
P = 128                # SBUF partitions
KT = C // P            # 4 contraction tiles
NT = N // P            # 32 node tiles
MT = R // P            # 4 row tiles per core
BR = R + 2             # payload rows per rank: xg rows + 2 bitcast deg rows

F32 = mybir.dt.float32
BF16 = mybir.dt.bfloat16
F8 = mybir.dt.float8e4
BF = mybir.dt.np(BF16)
F8NP = mybir.dt.np(F8)
DR = mybir.MatmulPerfMode.DoubleRow

_cache = {}


def _build():
    nc = bacc.Bacc("TRN2", target_bir_lowering=False, debug=False,
                   num_devices=N_CORES)

    xT8 = nc.dram_tensor("xT8", [C, N], F8, kind="ExternalInput")      # x^T, full
    xTs8 = nc.dram_tensor("xTs8", [C, R], F8, kind="ExternalInput")    # own cols
    adjW8 = nc.dram_tensor("adjW8", [C, C], F8, kind="ExternalInput")
    xTs = nc.dram_tensor("xTs", [C, R], BF16, kind="ExternalInput")
    gcnW = nc.dram_tensor("gcnW", [C, C], BF16, kind="ExternalInput")
    bias = nc.dram_tensor("bias", [1, C], BF16, kind="ExternalInput")
    out = nc.dram_tensor("out", [R, C], F32, kind="ExternalOutput")

    rg = [list(range(N_CORES))]

    with tile.TileContext(nc) as tc:
        with (
            tc.tile_pool(name="sb", bufs=1) as sb,
            tc.tile_pool(name="sbo", bufs=2) as sbo,
            tc.tile_pool(name="dram", bufs=1, space="DRAM") as dram,
            tc.tile_pool(name="ps_a", bufs=1, space="PSUM") as ps_a,
            tc.tile_pool(name="ps_adj", bufs=2, space="PSUM") as ps_adj,
            tc.tile_pool(name="ps_deg", bufs=1, space="PSUM") as ps_deg,
            tc.tile_pool(name="ps_fin", bufs=2, space="PSUM") as ps_fin,
        ):
            # ---- input loads ------------------------------------------------
            bias_sb = sb.tile([1, C], BF16, name="bias_sb", tag="bias_sb")
            nc.sync.dma_start(bias_sb[:, :], bias[:, :])
            xTs_sb = [sb.tile([P, R], BF16, name=f"xTs{k}", tag=f"xTs{k}") for k in range(KT)]
            gcnW_sb = [sb.tile([P, C], BF16, name=f"gcnW{k}", tag=f"gcnW{k}") for k in range(KT)]
            # fp8 operands in DoubleRow layout [P, k-subtile, free]
            adjW8_sb = sb.tile([P, KT, C], F8, name="adjW8_sb", tag="adjW8_sb")
            xTs8_sb = sb.tile([P, KT, R], F8, name="xTs8_sb", tag="xTs8_sb")
            xT8_sb = sb.tile([P, KT, N], F8, name="xT8_sb", tag="xT8_sb")
            ones_col = sb.tile([P, 1], BF16, name="ones_col", tag="ones_col")
            scr = sb.tile([1, 8], F32, name="scr", tag="scr")

            for k in range(KT):
                nc.sync.dma_start(xTs_sb[k][:, :], xTs[P * k:P * (k + 1), :])
                nc.sync.dma_start(gcnW_sb[k][:, :], gcnW[P * k:P * (k + 1), :])
            for k in range(KT):
                nc.sync.dma_start(adjW8_sb[:, k, :], adjW8[P * k:P * (k + 1), :])
                nc.sync.dma_start(xTs8_sb[:, k, :], xTs8[P * k:P * (k + 1), :])
            for k in range(KT):
                nc.sync.dma_start(xT8_sb[:, k, :], xT8[P * k:P * (k + 1), :])
            nc.vector.memset(ones_col[:, :], 1.0)
            # preload the DVE reciprocal / ACT sqrt lookup tables off the
            # critical path (first use otherwise costs ~1.3us each)
            nc.vector.memset(scr[:, 0:4], 4.0)
            nc.vector.reciprocal(scr[:, 4:8], scr[:, 0:4])
            nc.scalar.sqrt(scr[:, 4:8], scr[:, 0:4])

            # ---- phase 1b: xg[r, f] = sum_c x[r, c] W_gcn[c, f] (own rows) --
            yb_in = dram.tile([R, C], BF16, name="yb_in", tag="yb_in")
            yb_out = dram.tile([N, C], BF16, addr_space="Shared",
                               name="yb_out", tag="yb_out")
            xg_sb = [sb.tile([P, C], BF16, name=f"xg{m}", tag=f"xg{m}") for m in range(MT)]
            for m in range(MT):
                pa = ps_a.tile([P, C], F32, name=f"psg{m}", tag="psa")
                for k in range(KT):
                    nc.tensor.matmul(pa[:, :],
                                     xTs_sb[k][:, P * m:P * (m + 1)],
                                     gcnW_sb[k][:, :],
                                     start=(k == 0), stop=(k == KT - 1))
                nc.vector.tensor_copy(xg_sb[m][:, :], pa[:, :])
                nc.gpsimd.dma_start(yb_in[P * m:P * (m + 1), :], xg_sb[m][:, :])

            # AllGather xg early: the collective stream is idle until the
            # rank-skew barrier clears (~60us), so this wire time is free.
            # NOTE: the y readback DMAs are issued AFTER phase 2 — an engine
            # queue hitting their AG1 wait before phase-2 compute would
            # freeze that engine's remaining phase-2 work.
            nc.gpsimd.collective_compute(
                "AllGather", mybir.AluOpType.bypass, replica_groups=rg,
                ins=[yb_in.opt()], outs=[yb_out.opt()])

            # ---- phase 1a: xwT[j, r] = sum_c W_adj[c, j] x^T[c, r]  (fp8 DR)
            xwT8_sb = sb.tile([P, KT, R], F8, name="xwT8_sb", tag="xwT8_sb")
            for j in range(KT):
                pa = ps_a.tile([P, R], F32, name=f"psa{j}", tag="psa")
                for k in range(0, KT, 2):
                    nc.tensor.matmul(pa[:, :],
                                     adjW8_sb[:, k:k + 2, P * j:P * (j + 1)],
                                     xTs8_sb[:, k:k + 2, :],
                                     start=(k == 0), stop=(k == KT - 2),
                                     perf_mode=DR)
                nc.vector.tensor_copy(xwT8_sb[:, j, :], pa[:, :])

            # ---- phase 2: adjT tiles (fp8 DR), mask (bf16), deg ------------
            mask_sb = [sb.tile([P, R], BF16, name=f"mask{t}", tag=f"mask{t}") for t in range(NT)]
            pdeg = ps_deg.tile([1, R], F32, name="pdeg", tag="pdeg")
            for t in range(NT):
                pt = ps_adj.tile([P, R], F32, name=f"psadj{t}", tag="psadj")
                for k in range(0, KT, 2):
                    nc.tensor.matmul(pt[:, :],
                                     xT8_sb[:, k:k + 2, P * t:P * (t + 1)],
                                     xwT8_sb[:, k:k + 2, :],
                                     start=(k == 0), stop=(k == KT - 2),
                                     perf_mode=DR)
                # mask split DVE (not_equal) / ACT (sign^2): DVE alone paces
                # phase 2 at ~27us; the split brings the wall to ~17us
                if t % 3 == 2:
                    nc.scalar.sign(mask_sb[t][:, :], pt[:, :])
                    nc.scalar.square(mask_sb[t][:, :], mask_sb[t][:, :])
                else:
                    nc.vector.tensor_scalar(mask_sb[t][:, :], pt[:, :], 0.0, None,
                                            mybir.AluOpType.not_equal)
                nc.tensor.matmul(pdeg[:, :], ones_col[:, :], mask_sb[t][:, :],
                                 start=(t == 0), stop=(t == NT - 1))

            deg_own = sb.tile([1, R], F32, name="deg_own", tag="deg_own")
            nc.vector.tensor_copy(deg_own[:, :], pdeg[:, :])

            # AllGather deg (the cross-core degree exchange).
            degb_in = dram.tile([R], F32, name="degb_in", tag="degb_in")
            degb_out = dram.tile([N], F32, addr_space="Shared", name="degb_out", tag="degb_out")
            # the deg bounce write + readbacks are the only gpsimd ring
            # entries besides the early xg bounce writes, so nothing
            # AG1-gated can be ordered ahead of them and delay the AG2
            # trigger (the y readbacks ride sync exclusively).
            # The payload is written PARTITION-MAJOR (j = 4p + t), so the
            # pre-AG2 write pays the strided transpose and the post-AG2
            # readbacks are fast contiguous [128, 4] reads.
            nc.gpsimd.dma_start(degb_in.rearrange("(p t) -> t p", t=MT), deg_own[:, :])
            nc.gpsimd.collective_compute(
                "AllGather", mybir.AluOpType.bypass, replica_groups=rg,
                ins=[degb_in.opt()], outs=[degb_out.opt()])

            # own-row readback FIRST on the ring: it is gated only on the
            # local degb write (~70us), so all the own-side dis math below
            # completes during AG2's wire time instead of queueing in the
            # post-AG2 window ahead of the y scalings
            deg_glob = sb.tile([P, NT], F32, name="deg_glob", tag="deg_glob")
            deg_ownp = sb.tile([P, MT], F32, name="deg_ownp", tag="deg_ownp")
            nc.gpsimd.dma_start(deg_ownp[:, :], degb_in.rearrange("(p t) -> p t", p=P))
            nc.gpsimd.dma_start(
                deg_glob[:, :].rearrange("p (i t) -> p i t", i=N_CORES),
                degb_out.rearrange("(i p t) -> p i t", i=N_CORES, p=P))

            dis_own = sb.tile([P, MT], F32, name="dis_own", tag="dis_own")
            nc.vector.reciprocal(dis_own[:, :], deg_ownp[:, :])
            nc.scalar.sqrt(dis_own[:, :], dis_own[:, :])
            # sqrt(deg) row-vector: cancels the dis_r row scaling for the bias.
            invdis_row = sb.tile([1, R], BF16, name="invdis_row", tag="invdis_row")
            nc.scalar.sqrt(invdis_row[:, :], deg_own[:, :])

            # y readbacks ride sync EXCLUSIVELY: any queue that also carries
            # phase-2 compute or the deg chain would hit these AG1-gated
            # waits first and stall that work (seen as 15-45us freezes).
            # Serialized delivery (~0.65us/tile) still outruns the final
            # matmul's per-tile consumption.
            y_mega = sb.tile([P, NT * C], BF16, name="y_mega", tag="y_mega")
            y_view = lambda t: y_mega[:, C * t:C * (t + 1)]
            for t in range(NT):
                nc.sync.dma_start(y_view(t), yb_out[P * t:P * (t + 1), :])

            # dis = deg^-1/2 (global, post-AG2)
            dis_glob = sb.tile([P, NT], F32, name="dis_glob", tag="dis_glob")
            nc.vector.reciprocal(dis_glob[:, :], deg_glob[:, :])
            nc.scalar.sqrt(dis_glob[:, :], dis_glob[:, :])

            # ---- phase 3: y *= dis; out_rows = dis_r * (A @ y) + b ----------
            # per-tile scalings split DVE:ACT 3:1 (measured 350ns vs 800ns
            # per tile).  Keeping some scales off DVE also matters for a
            # subtler reason: with everything on one engine Tile coalesces
            # that engine's wait before the reciprocal into one covering ALL
            # the y DMAs, adding ~4us to the dis chain.
            for t in range(NT):
                if t % 4 == 3:
                    nc.scalar.mul(y_view(t), y_view(t), dis_glob[:, t:t + 1])
                else:
                    nc.vector.tensor_scalar(y_view(t), y_view(t),
                                            dis_glob[:, t:t + 1], None,
                                            mybir.AluOpType.mult)

            # m-outer: each PSUM bank accumulates a long 33-matmul chain
            # (bank-cycling per matmul triggers the HAM oscillation mode)
            for m in range(MT):
                pf = ps_fin.tile([P, C], F32, name=f"psf{m}", tag="psf")
                for t in range(NT):
                    nc.tensor.matmul(pf[:, :],
                                     mask_sb[t][:, P * m:P * (m + 1)],
                                     y_view(t),
                                     start=(t == 0), stop=False)
                # += sqrt(deg_r) (x) bias  — cancels against the dis_r scaling
                nc.tensor.matmul(pf[:, :],
                                 invdis_row[:, P * m:P * (m + 1)],
                                 bias_sb[:, :],
                                 start=False, stop=True)
                ot = sbo.tile([P, C], F32, name=f"outt{m}", tag="outt")
                nc.vector.tensor_scalar(ot[:, :], pf[:, :], dis_own[:, m:m + 1],
                                        None, mybir.AluOpType.mult)
                nc.sync.dma_start(out[P * m:P * (m + 1), :], ot[:, :])

    nc.compile()
    return nc


def _get_nc():
    if "nc" not in _cache:
        _cache["nc"] = _build()
    return _cache["nc"]


def _run(inputs, trace=False, trace_cores=None):
    x = np.asarray(inputs["x"], dtype=np.float32)
    adj_weight = np.asarray(inputs["adj_weight"], dtype=np.float32)
    gcn_weight = np.asarray(inputs["gcn_weight"], dtype=np.float32)
    gcn_bias = np.asarray(inputs["gcn_bias"], dtype=np.float32)

    xT = np.ascontiguousarray(x.T)                     # [C, N] f32
    xT8 = xT.astype(F8NP)
    adjW8 = adj_weight.astype(F8NP)
    gcnW = gcn_weight.astype(BF)
    bias_bf = gcn_bias.reshape(1, C).astype(BF)

    in_maps = []
    for i in range(N_CORES):
        sl = xT[:, R * i:R * (i + 1)]
        in_maps.append({
            "xT8": xT8,
            "xTs8": np.ascontiguousarray(xT8[:, R * i:R * (i + 1)]),
            "adjW8": adjW8,
            "xTs": np.ascontiguousarray(sl).astype(BF),
            "gcnW": gcnW,
            "bias": bias_bf,
        })

    nc = _get_nc()
    res = run_bass_kernel_spmd(nc, in_maps, core_ids=list(range(N_CORES)),
                               trace=trace, trace_cores=trace_cores)
    full = np.concatenate([res.results[i]["out"] for i in range(N_CORES)], axis=0)
    return full, res


def kernel(**inputs):
    full, _ = _run(inputs, trace=False)
    return full

